# revision 29
# baseline (speedup 1.0000x reference)
"""Trainium2 Bass kernel for nn_ASDSSMWrapper (Mamba-S6 selective SSM wrapper).

Computation (reference):
  hidden = x + x_res                      # [N,L,C] = [128,512,64]
  flatten T = N*L = 65536 tokens
  xz = hidden @ W_in; xi = silu(xz[:, :128]); z = xz[:, 128:]
  xdb = xi @ W_x -> dt_r[4], B[8], C[8]
  dt = softplus(dt_r @ W_dt + b_dt)       # [T, 128]
  a = exp(dt[:,:,None] * A[None])         # [T,128,8], A = -exp(A_log)
  b = (dt*xi)[:,:,None] * B[:,None,:]
  h_t = a_t h_{t-1} + b_t  (scan over all T, h_0 = 0)
  y = einsum('tds,ts->td', h, C) + D*xi; y = y * silu(z)
  out = y @ W_out; x_out = out.reshape + hidden; return (x_out, hidden)

Sharding: token axis split over 8 cores (8192 tokens each), no cross-core
state halo (state influence decays as exp(-dt*s*n); measured no-halo error
~3e-7, far below wire precision).

Host computes hidden = x + x_res exactly (it is output[1] and the residual
for output[0]); the device computes only the small SSM correction
out = ssm(hidden) (magnitude ~3e-2), which the host adds back to f32 hidden.
Wire format is fp16 both ways in a pre-transposed [C, T] layout, so
quantization error rides on the correction only (end-to-end rel err ~1e-4).

Engine economics measured on HW (per [128,2048] plane): DVE tensor_tensor
f32/psum 2.3us, fp16 2x-mode 1.2us (bf16 has NO 2x uop - avoid), DVE scan
4.5us (2 cyc/elem, any dtype, DVE-only; POOL cannot scan), ACT op 2.0us,
POOL tensor_tensor 4-6us AND pool steals a DVE SBUF port (exclusive lock),
halving DVE while active - so POOL is left idle. Assignment:
  PE (fp16): projections + one-hot e_mat matmuls broadcasting B_s/C_s rows
    of xdb across partitions (into PSUM).
  ACT: silu(xi), silu(z), softplus via exp+ln (one LUT set with exp; table
    switches cost 1.28us so the two silus are kept adjacent), 8x
    exp(A_s*dt) hoisted ahead of the state loop, PSUM->SBUF fp16 staging of
    C_s planes (enables fp16 2x h*C on DVE), xiD, output downconvert.
  DVE (the bottleneck, ~97% busy in steady state): b_s = dtxi * B_bc
    (PSUM operand, 1x), 8 scans (fp16 storage, fp32 internal state),
    h_s * C_s (fp16 2x), running y accumulation, yg, chain copies.
Steady-state period per state ~9.6us: b 2.3 + scan 4.5 + h*C 1.2 + acc 1.2.
PSUM: broadcast/out planes [128,2048] (4 banks, bufs=1) + projection chunks
[*,1024] (2 banks, bufs=2) so next-tile projections (issued with
tc.high_priority) overlap the state loop. Per-tile input/output DMAs and
w_in-first weight order shrink the startup ramp.
"""

import os
import tempfile

import numpy as np

import concourse.bass as bass
import concourse.tile as tile
import concourse.mybir as mybir

# Persist compiled executables (incl. the embedded NEFF) across processes so
# a fresh-process first call skips the multi-second walrus compile.
try:
    import jax as _jax
    _jax.config.update("jax_compilation_cache_dir",
                       os.path.join(tempfile.gettempdir(), "jax_cache_asdssm"))
    _jax.config.update("jax_persistent_cache_min_entry_size_bytes", 0)
    _jax.config.update("jax_persistent_cache_min_compile_time_secs", 0.5)
except Exception:
    pass

F32 = mybir.dt.float32
F16 = mybir.dt.float16
AF = mybir.ActivationFunctionType
OP = mybir.AluOpType

N, L, C = 128, 512, 64
D_INNER = 128          # EXPAND * C
DT_RANK = 4
S = 8                  # D_STATE
T = N * L              # 65536
NCORES = 8
TCORE = T // NCORES    # 8192 tokens per core, no halo
TILE_T = 2048          # tokens per on-chip tile
NT = TCORE // TILE_T   # 4 tiles
MM = 512               # matmul moving-operand chunk (walrus ISA limit)

_cache = {}


def _split_excess_waits(nc):
    """This walrus build allows 1 sync wait per instruction (2 for EventSem);
    hoist excess waits onto NoOps inserted just before the instruction."""
    for func in nc.m.functions:
        for block in func.blocks:
            out, changed = [], False
            for inst in block.instructions:
                si = inst.sync_info
                waits = list(si.on_wait) if si is not None and si.on_wait else []
                if len(waits) > 1:
                    for w in waits[:-1]:
                        nop = mybir.InstNoOp(
                            name=nc.get_next_instruction_name(), ins=[], outs=[])
                        nop.engine = inst.engine
                        nop.sync_info = mybir.SyncInfo(on_wait=[w], on_update=[])
                        out.append(nop)
                    si.on_wait = [waits[-1]]
                    inst.sync_info = si
                    changed = True
                out.append(inst)
            if changed:
                block.instructions = out


def _build():
    nc = bass.Bass()

    hid_in = nc.dram_tensor("hidT", [C, TCORE], F16, kind="ExternalInput")
    w_in = nc.dram_tensor("w_in", [C, 2 * D_INNER], F16, kind="ExternalInput")
    w_x = nc.dram_tensor("w_x", [D_INNER, DT_RANK + 2 * S], F16, kind="ExternalInput")
    w_dt = nc.dram_tensor("w_dt", [DT_RANK, D_INNER], F16, kind="ExternalInput")
    b_dt = nc.dram_tensor("b_dt", [D_INNER, 1], F32, kind="ExternalInput")
    a_mat = nc.dram_tensor("a_mat", [D_INNER, S], F32, kind="ExternalInput")
    d_vec = nc.dram_tensor("d_vec", [D_INNER, 1], F32, kind="ExternalInput")
    w_out = nc.dram_tensor("w_out", [D_INNER, C], F16, kind="ExternalInput")
    e_mat = nc.dram_tensor("e_mat", [DT_RANK + 2 * S, 16 * 128], F16,
                           kind="ExternalInput")

    out_t = nc.dram_tensor("outT", [C, TCORE], F16, kind="ExternalOutput")

    import contextlib as _ctx
    with tile.TileContext(nc) as tc:
        with _ctx.ExitStack() as _stk:
            def _pool(**kw):
                return _stk.enter_context(tc.tile_pool(**kw))
            consts = _pool(name="consts", bufs=1)
            slab_io = _pool(name="slab_io", bufs=1)
            xip = _pool(name="xip", bufs=2)
            dtp = _pool(name="dtp", bufs=2)
            xdbp = _pool(name="xdbp", bufs=2)
            szp = _pool(name="szp", bufs=2)
            ap_ = _pool(name="ap_", bufs=5)
            bp = _pool(name="bp", bufs=2)
            hp = _pool(name="hp", bufs=9)
            cstg = _pool(name="cstg", bufs=2)
            tmpp = _pool(name="tmp", bufs=4)
            prp = _pool(name="prp", bufs=2)
            xidp = _pool(name="xidp", bufs=2)
            ytree = _pool(name="ytree", bufs=1)
            chainp = _pool(name="chainp", bufs=2)
            ps_mm = _pool(name="ps_mm", bufs=2, space="PSUM")
            ps_bc = _pool(name="ps_bc", bufs=1, space="PSUM")
            # ---- weights + input slab; w_in and the first input tile come
            # first so the xi matmuls start as early as possible ----
            w_in_sb = consts.tile([C, 2 * D_INNER], F16)
            nc.sync.dma_start(out=w_in_sb, in_=w_in[:, :])
            hp_sb = slab_io.tile([C, TCORE], F16)
            _off = 0
            for _w in (1024, 1024, 2048, 2048, 2048):
                nc.sync.dma_start(out=hp_sb[:, _off:_off + _w],
                                  in_=hid_in[:, _off:_off + _w])
                _off += _w
            w_x_sb = consts.tile([D_INNER, DT_RANK + 2 * S], F16)
            nc.sync.dma_start(out=w_x_sb, in_=w_x[:, :])
            w_dt_sb = consts.tile([DT_RANK, D_INNER], F16)
            nc.sync.dma_start(out=w_dt_sb, in_=w_dt[:, :])
            bdt_sb = consts.tile([D_INNER, 1], F32)
            nc.sync.dma_start(out=bdt_sb, in_=b_dt[:, :])
            a_sb = consts.tile([D_INNER, S], F32)
            nc.sync.dma_start(out=a_sb, in_=a_mat[:, :])
            d_sb = consts.tile([D_INNER, 1], F32)
            nc.sync.dma_start(out=d_sb, in_=d_vec[:, :])
            w_out_sb = consts.tile([D_INNER, C], F16)
            nc.sync.dma_start(out=w_out_sb, in_=w_out[:, :])
            # e_mat: one-hot rows that broadcast xdb row DT_RANK+i across
            # 128 partitions via PE; uploaded once (device-resident weight).
            e_sb = consts.tile([DT_RANK + 2 * S, 16 * 128], F16)
            nc.sync.dma_start(out=e_sb, in_=e_mat[:, :])
            outp_sb = slab_io.tile([C, TCORE], F16)

            def mmW(parts, lhsT, rhs_fn, W):
                """[parts, min(W,1024)] psum chunks for one logical W-col mm."""
                PW = min(W, 1024)
                tiles = []
                for h in range(W // PW):
                    t_ = ps_mm.tile([parts, PW], F32, tag="mm")
                    for c in range(PW // MM):
                        col = h * PW + c * MM
                        nc.tensor.matmul(t_[:, c * MM:(c + 1) * MM],
                                         lhsT, rhs_fn(col),
                                         start=True, stop=True)
                    tiles.append((t_, h * PW, PW))
                return tiles

            import contextlib
            # narrower leading segments shorten the projection-ladder
            # latency to the first scans (tile-0 ramp); SBUF tiles stay at
            # full TILE_T width, ops just use the first W columns
            SEGS = []
            off = 0
            for W in (1024, 1024, 2048, 2048, 2048):
                SEGS.append((off, W)); off += W
            assert off == TCORE
            W_prev = None
            for j, (base, W) in enumerate(SEGS):
                bsl = slice(base, base + W)
                # pull this tile's projection/dt/exp phase ahead so it
                # overlaps the previous tile's state loop
                prio = tc.high_priority(offset=290) if j > 0 else contextlib.nullcontext()
                prio.__enter__()

                # ---- projections (psum chunks, consumers per-chunk)
                xi16 = xip.tile([D_INNER, TILE_T], F16, tag="xi16")
                for t_, hoff, PW in mmW(D_INNER, w_in_sb[:, 0:D_INNER],
                                        lambda col: hp_sb[:, base + col:base + col + MM], W):
                    nc.scalar.activation(xi16[:, hoff:hoff + PW], t_, AF.Silu)

                # z-branch silu adjacent to silu_xi: same ACT table
                sz16 = szp.tile([D_INNER, TILE_T], F16, tag="sz")
                for t_, hoff, PW in mmW(D_INNER, w_in_sb[:, D_INNER:2 * D_INNER],
                                        lambda col: hp_sb[:, base + col:base + col + MM], W):
                    nc.scalar.activation(sz16[:, hoff:hoff + PW], t_, AF.Silu)

                xdb16 = xdbp.tile([DT_RANK + 2 * S, TILE_T], F16, tag="xdb")
                for t_, hoff, PW in mmW(DT_RANK + 2 * S, w_x_sb,
                                        lambda col: xi16[:, col:col + MM], W):
                    nc.scalar.copy(out=xdb16[:, hoff:hoff + PW], in_=t_)

                # softplus(v + b_dt) = ln(1 + exp(v + b_dt)); exp and ln share
                # one ACT table set (natural_log_exp_and_others)
                dt_f = dtp.tile([D_INNER, TILE_T], F16, tag="dt")
                for t_, hoff, PW in mmW(D_INNER, w_dt_sb,
                                        lambda col: xdb16[0:DT_RANK, col:col + MM], W):
                    nc.scalar.activation(dt_f[:, hoff:hoff + PW], t_,
                                         AF.Exp, bias=bdt_sb[:, 0:1])
                nc.scalar.activation(dt_f[:, :W], dt_f[:, :W], AF.Ln, bias=1.0)

                # all 8 per-state decay planes hoisted ahead of the state loop
                # so the scans never wait on ACT
                a_tiles = []
                for s in range(S):
                    a_t = ap_.tile([D_INNER, TILE_T], F16, tag="a")
                    nc.scalar.activation(a_t[:, :W], dt_f[:, :W], AF.Exp,
                                         scale=a_sb[:, s:s + 1])
                    a_tiles.append(a_t)

                dtxi = dtp.tile([D_INNER, TILE_T], F16, tag="dtxi")
                nc.vector.tensor_tensor(out=dtxi[:, :W], in0=dt_f[:, :W],
                                        in1=xi16[:, :W], op=OP.mult)

                # xiD = D * xi (seeds the running y sum)
                xiD = xidp.tile([D_INNER, TILE_T], F16, tag="xiD")
                nc.vector.tensor_scalar_mul(xiD[:, :W], xi16[:, :W], d_sb[:, 0:1])
                ysum = xiD
                prio.__exit__(None, None, None)

                # ---- per-state: broadcast B/C, build b, scan, h*C ----
                h_cur = [None] * S
                for s in range(S):
                    bbc_ps = ps_bc.tile([D_INNER, TILE_T], F32, tag="bc")
                    for c in range(W // MM):
                        nc.tensor.matmul(bbc_ps[:, c * MM:(c + 1) * MM],
                                         e_sb[:, s * 128:(s + 1) * 128],
                                         xdb16[:, c * MM:(c + 1) * MM],
                                         start=True, stop=True)
                    b_t = bp.tile([D_INNER, TILE_T], F16, tag="b")
                    nc.vector.tensor_tensor(out=b_t[:, :W], in0=dtxi[:, :W],
                                            in1=bbc_ps[:, :W], op=OP.mult)

                    h_t = hp.tile([D_INNER, TILE_T], F16, tag="h")
                    init = 0.0 if j == 0 else h_prev[s][:, W_prev - 1:W_prev]
                    nc.vector.tensor_tensor_scan(
                        out=h_t[:, :W], data0=a_tiles[s][:, :W],
                        data1=b_t[:, :W], initial=init,
                        op0=OP.mult, op1=OP.add)
                    h_cur[s] = h_t

                    cbc_ps = ps_bc.tile([D_INNER, TILE_T], F32, tag="bc")
                    for c in range(W // MM):
                        nc.tensor.matmul(cbc_ps[:, c * MM:(c + 1) * MM],
                                         e_sb[:, (S + s) * 128:(S + s + 1) * 128],
                                         xdb16[:, c * MM:(c + 1) * MM],
                                         start=True, stop=True)
                    c16 = cstg.tile([D_INNER, TILE_T], F16, tag="cstg")
                    nc.scalar.copy(out=c16[:, :W], in_=cbc_ps[:, :W])
                    # POOL shares SBUF ports with DVE (exclusive lock) so
                    # pool offload halves DVE throughput - keep everything
                    # on DVE in fp16 2x mode instead.
                    tmp_t = tmpp.tile([D_INNER, TILE_T], F16, tag="hc")
                    nc.vector.tensor_tensor(out=tmp_t[:, :W], in0=h_t[:, :W],
                                            in1=c16[:, :W], op=OP.mult)

                    # single running sum on DVE seeded with xiD (ready
                    # early), so the tile-end tail is just the final add + yg
                    acc = prp.tile([D_INNER, TILE_T], F16, tag="acc0")
                    nc.vector.tensor_tensor(out=acc[:, :W], in0=ysum[:, :W],
                                            in1=tmp_t[:, :W], op=OP.add)
                    ysum = acc
                h_prev = h_cur
                W_prev = W

                # ---- yg = y * silu(z); out = W_out.T @ yg ----
                yg16 = ytree.tile([D_INNER, TILE_T], F16, tag="yg")
                nc.vector.tensor_tensor(out=yg16[:, :W], in0=ysum[:, :W],
                                        in1=sz16[:, :W], op=OP.mult)

                out_ps = ps_bc.tile([C, TILE_T], F32, tag="bc")
                for c in range(W // MM):
                    nc.tensor.matmul(out_ps[:, c * MM:(c + 1) * MM],
                                     w_out_sb, yg16[:, c * MM:(c + 1) * MM],
                                     start=True, stop=True)
                nc.scalar.copy(out=outp_sb[:, bsl], in_=out_ps[:, :W])
                nc.sync.dma_start(out=out_t[:, bsl], in_=outp_sb[:, bsl])

    _split_excess_waits(nc)
    return nc


def _get_runner():
    if "runner" in _cache:
        return _cache["runner"]
    import jax
    from jax.sharding import Mesh, PartitionSpec
    from jax.experimental.shard_map import shard_map
    from concourse.bass2jax import (
        _bass_exec_p, install_neuronx_cc_hook, partition_id_tensor)

    install_neuronx_cc_hook()
    nc = _build()
    _cache["nc"] = nc

    partition_name = nc.partition_id_tensor.name if nc.partition_id_tensor else None
    in_names, out_names, out_avals = [], [], []
    for alloc in nc.m.functions[0].allocations:
        if not isinstance(alloc, mybir.MemoryLocationSet):
            continue
        assert alloc.memorylocations
        name = alloc.memorylocations[0].name
        if alloc.kind == "ExternalInput":
            if name != partition_name:
                in_names.append(name)
        elif alloc.kind == "ExternalOutput":
            out_names.append(name)
            out_avals.append(jax.core.ShapedArray(
                tuple(alloc.tensor_shape), mybir.dt.np(alloc.dtype)))
    n_params = len(in_names)
    if partition_name is not None:
        in_names = in_names + [partition_name]

    def _body(*args):
        operands = list(args)
        if partition_name is not None:
            operands.append(partition_id_tensor())
        outs = _bass_exec_p.bind(
            *operands,
            out_avals=tuple(out_avals),
            in_names=tuple(in_names),
            out_names=tuple(out_names),
            lowering_input_output_aliases=(),
            sim_require_finite=True,
            sim_require_nnan=True,
            nc=nc,
        )
        return tuple(outs)

    devices = jax.devices()[:NCORES]
    assert len(devices) == NCORES
    mesh = Mesh(np.asarray(devices), ("core",))
    _cache["mesh"] = mesh
    sharded = jax.jit(
        shard_map(
            _body, mesh=mesh,
            in_specs=(PartitionSpec("core"),) * n_params,
            out_specs=(PartitionSpec("core"),) * len(out_names),
            check_rep=False,
        ),
        keep_unused=True,
    )
    _cache["runner"] = (sharded, in_names[:n_params], out_names)
    return _cache["runner"]


def _get_host_jits():
    """XLA-CPU kernels for the host-side pre/post passes."""
    if "host_jits" in _cache:
        return _cache["host_jits"]
    import jax
    import jax.numpy as jnp
    cpu = jax.devices("cpu")[0]

    @(lambda f: jax.jit(f, device=cpu))
    def pre(xa, xb):
        hidden = xa + xb                                       # [N,L,C] f32
        hT = jnp.transpose(hidden.reshape(NCORES, TCORE, C), (0, 2, 1))
        hT = hT.astype(jnp.float16).reshape(NCORES * C, TCORE)
        return hidden, hT

    @(lambda f: jax.jit(f, device=cpu))
    def post(o_f16, hidden):
        o32 = o_f16.astype(jnp.float32).reshape(NCORES, C, TCORE)
        o32 = jnp.transpose(o32, (0, 2, 1)).reshape(N, L, C)
        return o32 + hidden

    _cache["host_jits"] = (pre, post)
    return _cache["host_jits"]


def _host_e_mat():
    # one-hot broadcast matrix: block i of 128 columns selects xdb row
    # DT_RANK+i onto every output partition
    e = np.zeros((DT_RANK + 2 * S, 16 * 128), np.float16)
    for i in range(2 * S):
        e[DT_RANK + i, i * 128:(i + 1) * 128] = 1.0
    return e


def kernel(x, x_res, scale_id=None, W_in=None, W_x=None, W_dt=None, b_dt=None,
           A_log=None, D=None, W_out=None, **_):
    x = np.asarray(x, np.float32)
    x_res = np.asarray(x_res, np.float32)
    n, l, c = x.shape
    assert (n, l, c) == (N, L, C), (n, l, c)

    pre, post = _get_host_jits()
    hidden, hT_all = pre(x, x_res)
    hT_np = np.asarray(hT_all)

    A = -np.exp(np.asarray(A_log, np.float32))           # [128, 8]
    per_core = dict(
        w_in=np.ascontiguousarray(np.asarray(W_in, np.float32).astype(np.float16)),
        w_x=np.ascontiguousarray(np.asarray(W_x, np.float32).astype(np.float16)),
        w_dt=np.ascontiguousarray(np.asarray(W_dt, np.float32).astype(np.float16)),
        b_dt=np.ascontiguousarray(np.asarray(b_dt, np.float32).reshape(D_INNER, 1)),
        a_mat=np.ascontiguousarray(A),
        d_vec=np.ascontiguousarray(np.asarray(D, np.float32).reshape(D_INNER, 1)),
        w_out=np.ascontiguousarray(np.asarray(W_out, np.float32).astype(np.float16)),
        e_mat=_host_e_mat(),
    )

    sharded, in_names, out_names = _get_runner()

    # Device-resident weight cache: weights are static across calls in
    # practice; verify cheaply (they total ~60 KB) and re-upload on change.
    wc = _cache.get("weights")
    if wc is not None and all(
            np.array_equal(per_core[k], wc[0][k]) for k in per_core):
        dev_weights = wc[1]
    else:
        import jax
        from jax.sharding import NamedSharding, PartitionSpec
        mesh = _cache["mesh"]
        sh = NamedSharding(mesh, PartitionSpec("core"))
        dev_weights = {
            k: jax.device_put(np.concatenate([v] * NCORES, axis=0), sh)
            for k, v in per_core.items()
        }
        _cache["weights"] = (per_core, dev_weights)

    global_ins = [hT_np if name == "hidT" else dev_weights[name]
                  for name in in_names]
    _cache["last_global_ins"] = global_ins

    if "warmed" not in _cache:
        for _ in range(2):
            np.asarray(sharded(*global_ins)[0])
        _cache["warmed"] = True

    out_arrs = sharded(*global_ins)                      # async dispatch

    hid_np = np.asarray(hidden)   # overlaps the device round trip
    o_f16 = np.asarray(out_arrs[0])                      # [NCORES*C, TCORE] f16
    x_out = np.asarray(post(o_f16, hidden))
    return (x_out, hid_np)


if __name__ == "__main__":
    nc = _build()
    print("build ok:", sum(len(b.instructions) for f in nc.m.functions for b in f.blocks), "instructions")


# revision 30
# speedup vs baseline: 1.0563x; 1.0563x over previous
"""Trainium2 Bass kernel for nn_ASDSSMWrapper (Mamba-S6 selective SSM wrapper).

Computation (reference):
  hidden = x + x_res                      # [N,L,C] = [128,512,64]
  flatten T = N*L = 65536 tokens
  xz = hidden @ W_in; xi = silu(xz[:, :128]); z = xz[:, 128:]
  xdb = xi @ W_x -> dt_r[4], B[8], C[8]
  dt = softplus(dt_r @ W_dt + b_dt)       # [T, 128]
  a = exp(dt[:,:,None] * A[None])         # [T,128,8], A = -exp(A_log)
  b = (dt*xi)[:,:,None] * B[:,None,:]
  h_t = a_t h_{t-1} + b_t  (scan over all T, h_0 = 0)
  y = einsum('tds,ts->td', h, C) + D*xi; y = y * silu(z)
  out = y @ W_out; x_out = out.reshape + hidden; return (x_out, hidden)

Sharding: token axis split over 8 cores (8192 tokens each), no cross-core
state halo (state influence decays as exp(-dt*s*n); measured no-halo error
~3e-7, far below wire precision).

Host computes hidden = x + x_res exactly (it is output[1] and the residual
for output[0]); the device computes only the small SSM correction
out = ssm(hidden) (magnitude ~3e-2), which the host adds back to f32 hidden.
Wire format is fp16 both ways in a pre-transposed [C, T] layout, so
quantization error rides on the correction only (end-to-end rel err ~1e-4).

Engine economics measured on HW (per [128,2048] plane): DVE tensor_tensor
f32/psum 2.3us, fp16 2x-mode 1.2us (bf16 has NO 2x uop - avoid), DVE scan
4.5us (2 cyc/elem, any dtype, DVE-only; POOL cannot scan), ACT op 2.0us,
POOL tensor_tensor 4-6us AND pool steals a DVE SBUF port (exclusive lock),
halving DVE while active - so POOL is left idle. Assignment:
  PE (fp16): projections + one-hot e_mat matmuls broadcasting B_s/C_s rows
    of xdb across partitions (into PSUM).
  ACT: silu(xi), silu(z), softplus via exp+ln (one LUT set with exp; table
    switches cost 1.28us so the two silus are kept adjacent), 8x
    exp(A_s*dt) hoisted ahead of the state loop, PSUM->SBUF fp16 staging of
    C_s planes (enables fp16 2x h*C on DVE), xiD, output downconvert.
  DVE (the bottleneck, ~97% busy in steady state): b_s = dtxi * B_bc
    (PSUM operand, 1x), 8 scans (fp16 storage, fp32 internal state),
    h_s * C_s (fp16 2x), running y accumulation, yg, chain copies.
Steady-state period per state ~9.6us: b 2.3 + scan 4.5 + h*C 1.2 + acc 1.2.
PSUM: broadcast/out planes [128,2048] (4 banks, bufs=1) + projection chunks
[*,1024] (2 banks, bufs=2) so next-tile projections (issued with
tc.high_priority) overlap the state loop. Per-tile input/output DMAs and
w_in-first weight order shrink the startup ramp.
"""

import os
import tempfile

import numpy as np

import concourse.bass as bass
import concourse.tile as tile
import concourse.mybir as mybir

# Persist compiled executables (incl. the embedded NEFF) across processes so
# a fresh-process first call skips the multi-second walrus compile.
try:
    import jax as _jax
    _jax.config.update("jax_compilation_cache_dir",
                       os.path.join(tempfile.gettempdir(), "jax_cache_asdssm"))
    _jax.config.update("jax_persistent_cache_min_entry_size_bytes", 0)
    _jax.config.update("jax_persistent_cache_min_compile_time_secs", 0.5)
except Exception:
    pass

F32 = mybir.dt.float32
F16 = mybir.dt.float16
AF = mybir.ActivationFunctionType
OP = mybir.AluOpType

N, L, C = 128, 512, 64
D_INNER = 128          # EXPAND * C
DT_RANK = 4
S = 8                  # D_STATE
T = N * L              # 65536
NCORES = 8
TCORE = T // NCORES    # 8192 tokens per core, no halo
TILE_T = 2048          # tokens per on-chip tile
NT = TCORE // TILE_T   # 4 tiles
MM = 512               # matmul moving-operand chunk (walrus ISA limit)

_cache = {}


def _split_excess_waits(nc):
    """This walrus build allows 1 sync wait per instruction (2 for EventSem);
    hoist excess waits onto NoOps inserted just before the instruction."""
    for func in nc.m.functions:
        for block in func.blocks:
            out, changed = [], False
            for inst in block.instructions:
                si = inst.sync_info
                waits = list(si.on_wait) if si is not None and si.on_wait else []
                if len(waits) > 1:
                    for w in waits[:-1]:
                        nop = mybir.InstNoOp(
                            name=nc.get_next_instruction_name(), ins=[], outs=[])
                        nop.engine = inst.engine
                        nop.sync_info = mybir.SyncInfo(on_wait=[w], on_update=[])
                        out.append(nop)
                    si.on_wait = [waits[-1]]
                    inst.sync_info = si
                    changed = True
                out.append(inst)
            if changed:
                block.instructions = out


def _build():
    nc = bass.Bass()

    hid_in = nc.dram_tensor("hidT", [C, TCORE], F16, kind="ExternalInput")
    w_in = nc.dram_tensor("w_in", [C, 2 * D_INNER], F16, kind="ExternalInput")
    w_x = nc.dram_tensor("w_x", [D_INNER, DT_RANK + 2 * S], F16, kind="ExternalInput")
    w_dt = nc.dram_tensor("w_dt", [DT_RANK, D_INNER], F16, kind="ExternalInput")
    b_dt = nc.dram_tensor("b_dt", [D_INNER, 1], F32, kind="ExternalInput")
    a_mat = nc.dram_tensor("a_mat", [D_INNER, S], F32, kind="ExternalInput")
    d_vec = nc.dram_tensor("d_vec", [D_INNER, 1], F32, kind="ExternalInput")
    w_out = nc.dram_tensor("w_out", [D_INNER, C], F16, kind="ExternalInput")
    e_mat = nc.dram_tensor("e_mat", [DT_RANK + 2 * S, 16 * 128], F16,
                           kind="ExternalInput")

    out_t = nc.dram_tensor("outT", [C, TCORE], F16, kind="ExternalOutput")

    import contextlib as _ctx
    with tile.TileContext(nc) as tc:
        with _ctx.ExitStack() as _stk:
            def _pool(**kw):
                return _stk.enter_context(tc.tile_pool(**kw))
            consts = _pool(name="consts", bufs=1)
            slab_io = _pool(name="slab_io", bufs=1)
            xip = _pool(name="xip", bufs=2)
            dtp = _pool(name="dtp", bufs=2)
            xdbp = _pool(name="xdbp", bufs=2)
            szp = _pool(name="szp", bufs=2)
            ap_ = _pool(name="ap_", bufs=5)
            bp = _pool(name="bp", bufs=2)
            hp = _pool(name="hp", bufs=9)
            cstg = _pool(name="cstg", bufs=2)
            tmpp = _pool(name="tmp", bufs=4)
            prp = _pool(name="prp", bufs=2)
            xidp = _pool(name="xidp", bufs=2)
            ytree = _pool(name="ytree", bufs=1)
            chainp = _pool(name="chainp", bufs=2)
            ps_mm = _pool(name="ps_mm", bufs=2, space="PSUM")
            ps_bc = _pool(name="ps_bc", bufs=1, space="PSUM")
            # ---- weights + input slab; w_in and the first input tile come
            # first so the xi matmuls start as early as possible ----
            w_in_sb = consts.tile([C, 2 * D_INNER], F16)
            nc.sync.dma_start(out=w_in_sb, in_=w_in[:, :])
            hp_sb = slab_io.tile([C, TCORE], F16)
            _off = 0
            for _w in (TILE_T,) * NT:
                nc.sync.dma_start(out=hp_sb[:, _off:_off + _w],
                                  in_=hid_in[:, _off:_off + _w])
                _off += _w
            w_x_sb = consts.tile([D_INNER, DT_RANK + 2 * S], F16)
            nc.sync.dma_start(out=w_x_sb, in_=w_x[:, :])
            w_dt_sb = consts.tile([DT_RANK, D_INNER], F16)
            nc.sync.dma_start(out=w_dt_sb, in_=w_dt[:, :])
            bdt_sb = consts.tile([D_INNER, 1], F32)
            nc.sync.dma_start(out=bdt_sb, in_=b_dt[:, :])
            a_sb = consts.tile([D_INNER, S], F32)
            nc.sync.dma_start(out=a_sb, in_=a_mat[:, :])
            d_sb = consts.tile([D_INNER, 1], F32)
            nc.sync.dma_start(out=d_sb, in_=d_vec[:, :])
            w_out_sb = consts.tile([D_INNER, C], F16)
            nc.sync.dma_start(out=w_out_sb, in_=w_out[:, :])
            # e_mat: one-hot rows that broadcast xdb row DT_RANK+i across
            # 128 partitions via PE; uploaded once (device-resident weight).
            e_sb = consts.tile([DT_RANK + 2 * S, 16 * 128], F16)
            nc.sync.dma_start(out=e_sb, in_=e_mat[:, :])
            outp_sb = slab_io.tile([C, TCORE], F16)

            def mmW(parts, lhsT, rhs_fn, W):
                """[parts, min(W,1024)] psum chunks for one logical W-col mm."""
                PW = min(W, 1024)
                tiles = []
                for h in range(W // PW):
                    t_ = ps_mm.tile([parts, PW], F32, tag="mm")
                    for c in range(PW // MM):
                        col = h * PW + c * MM
                        nc.tensor.matmul(t_[:, c * MM:(c + 1) * MM],
                                         lhsT, rhs_fn(col),
                                         start=True, stop=True)
                    tiles.append((t_, h * PW, PW))
                return tiles

            import contextlib
            # narrower leading segments shorten the projection-ladder
            # latency to the first scans (tile-0 ramp); SBUF tiles stay at
            # full TILE_T width, ops just use the first W columns
            SEGS = []
            off = 0
            for W in (TILE_T,) * NT:
                SEGS.append((off, W)); off += W
            assert off == TCORE
            W_prev = None
            for j, (base, W) in enumerate(SEGS):
                bsl = slice(base, base + W)
                # pull this tile's projection/dt/exp phase ahead so it
                # overlaps the previous tile's state loop
                prio = tc.high_priority(offset=290) if j > 0 else contextlib.nullcontext()
                prio.__enter__()

                # ---- projections (psum chunks, consumers per-chunk)
                xi16 = xip.tile([D_INNER, TILE_T], F16, tag="xi16")
                for t_, hoff, PW in mmW(D_INNER, w_in_sb[:, 0:D_INNER],
                                        lambda col: hp_sb[:, base + col:base + col + MM], W):
                    nc.scalar.activation(xi16[:, hoff:hoff + PW], t_, AF.Silu)

                # z-branch silu adjacent to silu_xi: same ACT table
                sz16 = szp.tile([D_INNER, TILE_T], F16, tag="sz")
                for t_, hoff, PW in mmW(D_INNER, w_in_sb[:, D_INNER:2 * D_INNER],
                                        lambda col: hp_sb[:, base + col:base + col + MM], W):
                    nc.scalar.activation(sz16[:, hoff:hoff + PW], t_, AF.Silu)

                xdb16 = xdbp.tile([DT_RANK + 2 * S, TILE_T], F16, tag="xdb")
                for t_, hoff, PW in mmW(DT_RANK + 2 * S, w_x_sb,
                                        lambda col: xi16[:, col:col + MM], W):
                    nc.scalar.copy(out=xdb16[:, hoff:hoff + PW], in_=t_)

                # softplus(v + b_dt) = ln(1 + exp(v + b_dt)); exp and ln share
                # one ACT table set (natural_log_exp_and_others)
                dt_f = dtp.tile([D_INNER, TILE_T], F16, tag="dt")
                for t_, hoff, PW in mmW(D_INNER, w_dt_sb,
                                        lambda col: xdb16[0:DT_RANK, col:col + MM], W):
                    nc.scalar.activation(dt_f[:, hoff:hoff + PW], t_,
                                         AF.Exp, bias=bdt_sb[:, 0:1])
                nc.scalar.activation(dt_f[:, :W], dt_f[:, :W], AF.Ln, bias=1.0)

                # all 8 per-state decay planes hoisted ahead of the state loop
                # so the scans never wait on ACT
                a_tiles = []
                for s in range(S):
                    a_t = ap_.tile([D_INNER, TILE_T], F16, tag="a")
                    nc.scalar.activation(a_t[:, :W], dt_f[:, :W], AF.Exp,
                                         scale=a_sb[:, s:s + 1])
                    a_tiles.append(a_t)

                dtxi = dtp.tile([D_INNER, TILE_T], F16, tag="dtxi")
                nc.vector.tensor_tensor(out=dtxi[:, :W], in0=dt_f[:, :W],
                                        in1=xi16[:, :W], op=OP.mult)

                # xiD = D * xi (seeds the running y sum)
                xiD = xidp.tile([D_INNER, TILE_T], F16, tag="xiD")
                nc.vector.tensor_scalar_mul(xiD[:, :W], xi16[:, :W], d_sb[:, 0:1])
                ysum = xiD
                prio.__exit__(None, None, None)

                # ---- per-state: broadcast B/C, build b, scan, h*C ----
                h_cur = [None] * S
                for s in range(S):
                    bbc_ps = ps_bc.tile([D_INNER, TILE_T], F32, tag="bc")
                    for c in range(W // MM):
                        nc.tensor.matmul(bbc_ps[:, c * MM:(c + 1) * MM],
                                         e_sb[:, s * 128:(s + 1) * 128],
                                         xdb16[:, c * MM:(c + 1) * MM],
                                         start=True, stop=True)
                    b_t = bp.tile([D_INNER, TILE_T], F16, tag="b")
                    nc.vector.tensor_tensor(out=b_t[:, :W], in0=dtxi[:, :W],
                                            in1=bbc_ps[:, :W], op=OP.mult)

                    h_t = hp.tile([D_INNER, TILE_T], F16, tag="h")
                    init = 0.0 if j == 0 else h_prev[s][:, W_prev - 1:W_prev]
                    nc.vector.tensor_tensor_scan(
                        out=h_t[:, :W], data0=a_tiles[s][:, :W],
                        data1=b_t[:, :W], initial=init,
                        op0=OP.mult, op1=OP.add)
                    h_cur[s] = h_t

                    cbc_ps = ps_bc.tile([D_INNER, TILE_T], F32, tag="bc")
                    for c in range(W // MM):
                        nc.tensor.matmul(cbc_ps[:, c * MM:(c + 1) * MM],
                                         e_sb[:, (S + s) * 128:(S + s + 1) * 128],
                                         xdb16[:, c * MM:(c + 1) * MM],
                                         start=True, stop=True)
                    c16 = cstg.tile([D_INNER, TILE_T], F16, tag="cstg")
                    nc.scalar.copy(out=c16[:, :W], in_=cbc_ps[:, :W])
                    # POOL shares SBUF ports with DVE (exclusive lock) so
                    # pool offload halves DVE throughput - keep everything
                    # on DVE in fp16 2x mode instead.
                    tmp_t = tmpp.tile([D_INNER, TILE_T], F16, tag="hc")
                    nc.vector.tensor_tensor(out=tmp_t[:, :W], in0=h_t[:, :W],
                                            in1=c16[:, :W], op=OP.mult)

                    # single running sum on DVE seeded with xiD (ready
                    # early), so the tile-end tail is just the final add + yg
                    acc = prp.tile([D_INNER, TILE_T], F16, tag="acc0")
                    nc.vector.tensor_tensor(out=acc[:, :W], in0=ysum[:, :W],
                                            in1=tmp_t[:, :W], op=OP.add)
                    ysum = acc
                h_prev = h_cur
                W_prev = W

                # ---- yg = y * silu(z); out = W_out.T @ yg ----
                yg16 = ytree.tile([D_INNER, TILE_T], F16, tag="yg")
                nc.vector.tensor_tensor(out=yg16[:, :W], in0=ysum[:, :W],
                                        in1=sz16[:, :W], op=OP.mult)

                out_ps = ps_bc.tile([C, TILE_T], F32, tag="bc")
                for c in range(W // MM):
                    nc.tensor.matmul(out_ps[:, c * MM:(c + 1) * MM],
                                     w_out_sb, yg16[:, c * MM:(c + 1) * MM],
                                     start=True, stop=True)
                nc.scalar.copy(out=outp_sb[:, bsl], in_=out_ps[:, :W])
                nc.sync.dma_start(out=out_t[:, bsl], in_=outp_sb[:, bsl])

    _split_excess_waits(nc)
    return nc


def _get_runner():
    if "runner" in _cache:
        return _cache["runner"]
    import jax
    from jax.sharding import Mesh, PartitionSpec
    from jax.experimental.shard_map import shard_map
    from concourse.bass2jax import (
        _bass_exec_p, install_neuronx_cc_hook, partition_id_tensor)

    install_neuronx_cc_hook()
    nc = _build()
    _cache["nc"] = nc

    partition_name = nc.partition_id_tensor.name if nc.partition_id_tensor else None
    in_names, out_names, out_avals = [], [], []
    for alloc in nc.m.functions[0].allocations:
        if not isinstance(alloc, mybir.MemoryLocationSet):
            continue
        assert alloc.memorylocations
        name = alloc.memorylocations[0].name
        if alloc.kind == "ExternalInput":
            if name != partition_name:
                in_names.append(name)
        elif alloc.kind == "ExternalOutput":
            out_names.append(name)
            out_avals.append(jax.core.ShapedArray(
                tuple(alloc.tensor_shape), mybir.dt.np(alloc.dtype)))
    n_params = len(in_names)
    if partition_name is not None:
        in_names = in_names + [partition_name]

    def _body(*args):
        operands = list(args)
        if partition_name is not None:
            operands.append(partition_id_tensor())
        outs = _bass_exec_p.bind(
            *operands,
            out_avals=tuple(out_avals),
            in_names=tuple(in_names),
            out_names=tuple(out_names),
            lowering_input_output_aliases=(),
            sim_require_finite=True,
            sim_require_nnan=True,
            nc=nc,
        )
        return tuple(outs)

    devices = jax.devices()[:NCORES]
    assert len(devices) == NCORES
    mesh = Mesh(np.asarray(devices), ("core",))
    _cache["mesh"] = mesh
    sharded = jax.jit(
        shard_map(
            _body, mesh=mesh,
            in_specs=(PartitionSpec("core"),) * n_params,
            out_specs=(PartitionSpec("core"),) * len(out_names),
            check_rep=False,
        ),
        keep_unused=True,
    )
    _cache["runner"] = (sharded, in_names[:n_params], out_names)
    return _cache["runner"]


def _get_host_jits():
    """XLA-CPU kernels for the host-side pre/post passes."""
    if "host_jits" in _cache:
        return _cache["host_jits"]
    import jax
    import jax.numpy as jnp
    cpu = jax.devices("cpu")[0]

    @(lambda f: jax.jit(f, device=cpu))
    def pre(xa, xb):
        hidden = xa + xb                                       # [N,L,C] f32
        hT = jnp.transpose(hidden.reshape(NCORES, TCORE, C), (0, 2, 1))
        hT = hT.astype(jnp.float16).reshape(NCORES * C, TCORE)
        return hidden, hT

    @(lambda f: jax.jit(f, device=cpu))
    def post(o_f16, hidden):
        o32 = o_f16.astype(jnp.float32).reshape(NCORES, C, TCORE)
        o32 = jnp.transpose(o32, (0, 2, 1)).reshape(N, L, C)
        return o32 + hidden

    _cache["host_jits"] = (pre, post)
    return _cache["host_jits"]


def _host_e_mat():
    # one-hot broadcast matrix: block i of 128 columns selects xdb row
    # DT_RANK+i onto every output partition
    e = np.zeros((DT_RANK + 2 * S, 16 * 128), np.float16)
    for i in range(2 * S):
        e[DT_RANK + i, i * 128:(i + 1) * 128] = 1.0
    return e


def kernel(x, x_res, scale_id=None, W_in=None, W_x=None, W_dt=None, b_dt=None,
           A_log=None, D=None, W_out=None, **_):
    x = np.asarray(x, np.float32)
    x_res = np.asarray(x_res, np.float32)
    n, l, c = x.shape
    assert (n, l, c) == (N, L, C), (n, l, c)

    pre, post = _get_host_jits()
    hidden, hT_all = pre(x, x_res)
    hT_np = np.asarray(hT_all)

    A = -np.exp(np.asarray(A_log, np.float32))           # [128, 8]
    per_core = dict(
        w_in=np.ascontiguousarray(np.asarray(W_in, np.float32).astype(np.float16)),
        w_x=np.ascontiguousarray(np.asarray(W_x, np.float32).astype(np.float16)),
        w_dt=np.ascontiguousarray(np.asarray(W_dt, np.float32).astype(np.float16)),
        b_dt=np.ascontiguousarray(np.asarray(b_dt, np.float32).reshape(D_INNER, 1)),
        a_mat=np.ascontiguousarray(A),
        d_vec=np.ascontiguousarray(np.asarray(D, np.float32).reshape(D_INNER, 1)),
        w_out=np.ascontiguousarray(np.asarray(W_out, np.float32).astype(np.float16)),
        e_mat=_host_e_mat(),
    )

    sharded, in_names, out_names = _get_runner()

    # Device-resident weight cache: weights are static across calls in
    # practice; verify cheaply (they total ~60 KB) and re-upload on change.
    wc = _cache.get("weights")
    if wc is not None and all(
            np.array_equal(per_core[k], wc[0][k]) for k in per_core):
        dev_weights = wc[1]
    else:
        import jax
        from jax.sharding import NamedSharding, PartitionSpec
        mesh = _cache["mesh"]
        sh = NamedSharding(mesh, PartitionSpec("core"))
        dev_weights = {
            k: jax.device_put(np.concatenate([v] * NCORES, axis=0), sh)
            for k, v in per_core.items()
        }
        _cache["weights"] = (per_core, dev_weights)

    global_ins = [hT_np if name == "hidT" else dev_weights[name]
                  for name in in_names]
    _cache["last_global_ins"] = global_ins

    if "warmed" not in _cache:
        for _ in range(2):
            np.asarray(sharded(*global_ins)[0])
        _cache["warmed"] = True

    out_arrs = sharded(*global_ins)                      # async dispatch

    hid_np = np.asarray(hidden)   # overlaps the device round trip
    o_f16 = np.asarray(out_arrs[0])                      # [NCORES*C, TCORE] f16
    x_out = np.asarray(post(o_f16, hidden))
    return (x_out, hid_np)


if __name__ == "__main__":
    nc = _build()
    print("build ok:", sum(len(b.instructions) for f in nc.m.functions for b in f.blocks), "instructions")
